# revision 19
# baseline (speedup 1.0000x reference)
"""Trainium2 Bass kernel for nn_DynamicHybridModulation.

Sharding: data-parallel over batch (B=8 -> 8 cores, one batch each).
The only cross-core communication is a 6-float AllReduce for the global
BatchNorm statistics of the bias branch.

Per-core dataflow (batch b, S=512, Dm=768, H=12, D=64, R=3):
  phase A: q'/k'/v projections.  q_lin^T / k_lin^T computed with W as the
           stationary operand (output lands transposed, [dout, s]); v_lin
           computed with hs^T stationary (output natural, [s, dout]).
           Biases folded in via an augmented contraction row (row 768 of the
           padded weights = bias, matching ones row in hs^T).
           q' = 0.5*spike(q_lin) (ternary, exact), k' = spike(k_lin)+k_lin.
           Note q'*k' == q*k of the reference, so stored scores are 8*ref.
  phase B: S'^T[k,q] = q'k' per head via K=64 matmuls (no transposes needed
           anywhere).  fp16 copy kept in SBUF; t = (S'>=8); row sums of t
           (over q, free axis) via tensor_scalar accum; column sums (over k)
           via ones-matmul on the PE.
  mid:     conv1 (rank-1 accumulated matmuls) -> local BN sums -> AllReduce
           -> BN -> relu -> convh/convw (+sigmoid) -> per-head broadcasts.
  phase C: pre = S' + 8*t*sh*sw via two fused scalar_tensor_tensor ops,
           probs_un = exp(pre/8 + mask) on ACT, ctx^T = V_aug^T @ probs_un
           with a ones column giving the softmax denominator for free,
           rows scaled by 1/denom on the way out.
Output returned as ctx^T [768, 512] per core; host transposes/stacks.
"""

import numpy as np

try:
    import concourse  # noqa: F401
except ImportError:  # pragma: no cover
    import sys

    for p in ("/opt/trn_rl_repo", "/root/.axon_site/_ro/trn_rl_repo"):
        sys.path.insert(0, p)

import concourse.bass as bass  # noqa: E402
import concourse.tile as tile  # noqa: E402
from concourse import bacc, mybir  # noqa: E402
from concourse.bass_utils import run_bass_kernel_spmd  # noqa: E402

F32 = mybir.dt.float32
F32R = mybir.dt.float32r
BF16 = mybir.dt.bfloat16
F16 = mybir.dt.float16
ALU = mybir.AluOpType
ACTF = mybir.ActivationFunctionType

B, S, DM, H, D, R = 8, 512, 768, 12, 64, 3
NT = DM // 128  # 6 dout tiles
KT = S // 128  # 4 s tiles
NI = NT + 1  # 7 contraction tiles (6 x 128 + bias row block)
N_TOT = float(B * 2 * S)  # BatchNorm normalizer (B * 2S)

_CACHE = {}


def _round_fp32r(x):
    """Round fp32 to the 11-explicit-mantissa-bit grid the PE uses for
    float32r operands (calibrated against hardware)."""
    u = np.ascontiguousarray(x, np.float32).view(np.uint32).copy()
    u = (u + np.uint32(0x800)) & np.uint32(0xFFFFF000)
    return u.view(np.float32)


def _build():
    nc = bacc.Bacc("TRN2", target_bir_lowering=False, debug=False, num_devices=8)

    # ---- DRAM I/O ----
    hsT_d = nc.dram_tensor("hsT", [NI * 128, S], F32R, kind="ExternalInput").ap()
    wq_d = nc.dram_tensor("wq", [NT, NI, 128, 128], F32R, kind="ExternalInput").ap()
    wk_d = nc.dram_tensor("wk", [NT, NI, 128, 128], F32R, kind="ExternalInput").ap()
    wv_d = nc.dram_tensor("wv", [NI, 128, DM], F32R, kind="ExternalInput").ap()
    maskT_d = nc.dram_tensor("maskT", [128, KT], F32, kind="ExternalInput").ap()
    conv1T_d = nc.dram_tensor("conv1T", [H, R], F32R, kind="ExternalInput").ap()
    convhT_d = nc.dram_tensor("convhT", [R, H], F32R, kind="ExternalInput").ap()
    convwT_d = nc.dram_tensor("convwT", [R, H], F32R, kind="ExternalInput").ap()
    gamma_d = nc.dram_tensor("gamma", [R, 1], F32, kind="ExternalInput").ap()
    beta_d = nc.dram_tensor("beta", [R, 1], F32, kind="ExternalInput").ap()
    outT_d = nc.dram_tensor("outT", [DM, S], F32, kind="ExternalOutput").ap()
    ar_in_d = nc.dram_tensor("ar_bounce", [R, 2], F32).ap()
    ar_out_d = nc.dram_tensor("ar_shared", [R, 2], F32, addr_space="Shared").ap()

    with tile.TileContext(nc) as tc:
        with (
            tc.tile_pool(name="const", bufs=1) as cpool,
            tc.tile_pool(name="wstream", bufs=3) as wpool,
            tc.tile_pool(name="big", bufs=1) as bigpool,
            tc.tile_pool(name="wk3", bufs=3) as wk3pool,
            tc.tile_pool(name="wk2", bufs=2) as wk2pool,
            tc.tile_pool(name="t16p", bufs=4) as t16pool,
            tc.tile_pool(name="shb", bufs=2) as shbpool,
            tc.tile_pool(name="ebuf", bufs=3) as epool,
            tc.tile_pool(name="ps", bufs=7, space="PSUM") as pspool,
        ):
            # ---- resident loads ----
            hsT_t = cpool.tile([128, NI, S], F32R)
            nc.sync.dma_start(hsT_t[:], hsT_d.rearrange("(i p) s -> p i s", p=128))
            wv_t = cpool.tile([128, NI, DM], F32R)
            nc.sync.dma_start(wv_t[:], wv_d.rearrange("i p c -> p i c"))
            maskT_t = cpool.tile([128, KT], F32)
            nc.sync.dma_start(maskT_t[:], maskT_d[:])
            conv1T_t = cpool.tile([H, R], F32R)
            nc.sync.dma_start(conv1T_t[:], conv1T_d[:])
            convhT_t = cpool.tile([R, H], F32R)
            nc.sync.dma_start(convhT_t[:], convhT_d[:])
            convwT_t = cpool.tile([R, H], F32R)
            nc.sync.dma_start(convwT_t[:], convwT_d[:])
            gamma_t = cpool.tile([R, 1], F32)
            nc.sync.dma_start(gamma_t[:], gamma_d[:])
            beta_t = cpool.tile([R, 1], F32)
            nc.sync.dma_start(beta_t[:], beta_d[:])

            ones_f16 = cpool.tile([128, 1], F16)
            nc.gpsimd.memset(ones_f16[:], 1.0)
            ones_row16 = cpool.tile([1, 128], F16)
            nc.gpsimd.memset(ones_row16[:], 1.0)
            ones_row_f = cpool.tile([1, D], F32)
            nc.gpsimd.memset(ones_row_f[:], 1.0)
            ones_row_r = cpool.tile([1, D], F32R)
            nc.scalar.copy(ones_row_r[:], ones_row_f[:])

            # ---- persistent intermediates ----
            qT_t = bigpool.tile([128, NT, S], F32R)  # 0.5*spike(q_lin)^T
            kT_t = bigpool.tile([128, NT, S], F32R)  # (spike+lin)(k_lin)^T
            v_t = bigpool.tile([128, KT, H * 65], BF16)  # v with ones cols
            s16_t = bigpool.tile([128, H * KT, S], F16)  # scores' (8x) fp16
            cat_t = bigpool.tile([H, 2 * S], F32R)  # [xh | xw] sums
            xw_cols = bigpool.tile([128, H * KT], F32R)  # accum slots
            sw8_cols = bigpool.tile([128, KT, H], F32)  # 8*sigmoid(convw...)
            sh_t = bigpool.tile([H, S], F16)

            # ones columns of v_t
            for st in range(KT):
                nc.gpsimd.memset(
                    v_t[:, st, :].rearrange("p (h c) -> p h c", c=65)[:, :, 64:65], 1.0
                )

            # =========== PHASE A: projections ===========
            # q/k: out^T via W stationary
            for proj, w_d, dst in (("q", wq_d, qT_t), ("k", wk_d, kT_t)):
                for j in range(NT):
                    w_t = wpool.tile([128, NI, 128], F32R, tag="wblk")
                    nc.sync.dma_start(w_t[:], w_d[j].rearrange("i p c -> p i c"))
                    pa = pspool.tile([128, S], F32, tag="ps")
                    for i in range(NI):
                        nc.tensor.matmul(
                            pa[:],
                            w_t[:, i, :],
                            hsT_t[:, i, :],
                            start=(i == 0),
                            stop=(i == NI - 1),
                        )
                    if proj == "q":
                        # q' = 0.5*(x>=1) - 0.5*(x<=-1)
                        t1 = wk3pool.tile([128, S], F32, tag="qk_tmp")
                        nc.vector.tensor_scalar(
                            t1[:], pa[:], 1.0, 0.5, ALU.is_ge, ALU.mult
                        )
                        t2 = wk3pool.tile([128, S], F32, tag="qk_tmp2")
                        nc.vector.tensor_scalar(
                            t2[:], pa[:], -1.0, -0.5, ALU.is_le, ALU.mult
                        )
                        nc.vector.tensor_tensor(
                            dst[:, j, :], t1[:], t2[:], ALU.add
                        )
                    else:
                        # k' = (x>=1) - (x<=-1) + x
                        t1 = wk3pool.tile([128, S], F32, tag="qk_tmp")
                        nc.vector.tensor_scalar(
                            t1[:], pa[:], -1.0, -1.0, ALU.is_le, ALU.mult
                        )
                        t2 = wk3pool.tile([128, S], F32, tag="qk_tmp2")
                        nc.vector.scalar_tensor_tensor(
                            t2[:], pa[:], 1.0, t1[:], ALU.is_ge, ALU.add
                        )
                        nc.vector.tensor_tensor(
                            dst[:, j, :], t2[:], pa[:], ALU.add
                        )

            # v: natural layout via hs^T stationary, write into aug layout
            for st in range(KT):
                for dh in range(2):
                    pv = pspool.tile([128, S], F32, tag="ps")
                    for i in range(NI):
                        nc.tensor.matmul(
                            pv[:, :384],
                            hsT_t[:, i, st * 128 : (st + 1) * 128],
                            wv_t[:, i, dh * 384 : (dh + 1) * 384],
                            start=(i == 0),
                            stop=(i == NI - 1),
                        )
                    # psum [128, 6 heads x 64] -> v_t aug cols (65 stride)
                    dst = v_t[:, st, dh * 6 * 65 : (dh + 1) * 6 * 65].rearrange(
                        "p (h c) -> p h c", c=65
                    )[:, :, 0:64]
                    nc.scalar.copy(dst, pv[:, :384].rearrange("p (h c) -> p h c", c=64))

            # =========== PHASE B: scores + t stats ===========
            for h in range(H):
                jh, p0 = divmod(h * D, 128)
                pxh = pspool.tile([1, S], F32, tag="ps")
                for kt in range(KT):
                    ps = pspool.tile([128, S], F32, tag="ps")
                    nc.tensor.matmul(
                        ps[:],
                        kT_t[p0 : p0 + D, jh, kt * 128 : (kt + 1) * 128],
                        qT_t[p0 : p0 + D, jh, :],
                        start=True,
                        stop=True,
                    )
                    s16 = s16_t[:, h * KT + kt, :]
                    nc.vector.tensor_copy(s16, ps[:])
                    t16 = t16pool.tile([128, S], F16, tag="t16")
                    nc.vector.tensor_scalar(
                        t16[:],
                        s16,
                        8.0,
                        None,
                        ALU.is_ge,
                        ALU.add,
                        accum_out=xw_cols[:, h * KT + kt : h * KT + kt + 1],
                    )
                    nc.tensor.matmul(
                        pxh[:],
                        ones_f16[:],
                        t16[:],
                        start=(kt == 0),
                        stop=(kt == KT - 1),
                    )
                # bounce PSUM -> SBUF (cast to f32r), then partition-shift DMA
                xh_row = wk2pool.tile([1, S], F32R, tag="xhrow")
                nc.scalar.copy(xh_row[:], pxh[:])
                nc.sync.dma_start(cat_t[h : h + 1, 0:S], xh_row[:])
                for kt in range(KT):
                    nc.sync.dma_start(
                        cat_t[h : h + 1, S + kt * 128 : S + (kt + 1) * 128],
                        xw_cols[:, h * KT + kt : h * KT + kt + 1],
                    )

            # =========== MID: conv1 -> BN(allreduce) -> gates ===========
            pyh = pspool.tile([R, S], F32, tag="ps")
            pyw = pspool.tile([R, S], F32, tag="ps")
            nc.tensor.matmul(pyh[:], conv1T_t[:], cat_t[:, 0:S], start=True, stop=True)
            nc.tensor.matmul(pyw[:], conv1T_t[:], cat_t[:, S:], start=True, stop=True)
            y_t = bigpool.tile([R, 2 * S], F32)
            nc.scalar.copy(y_t[:, :S], pyh[:])
            nc.scalar.copy(y_t[:, S:], pyw[:])

            stats_t = bigpool.tile([R, 2], F32)
            nc.vector.tensor_reduce(
                stats_t[:, 0:1], y_t[:], mybir.AxisListType.X, ALU.add
            )
            sq_t = bigpool.tile([R, 2 * S], F32)
            nc.vector.tensor_tensor(sq_t[:], y_t[:], y_t[:], ALU.mult)
            nc.vector.tensor_reduce(
                stats_t[:, 1:2], sq_t[:], mybir.AxisListType.X, ALU.add
            )
            nc.sync.dma_start(ar_in_d[:], stats_t[:])
            nc.gpsimd.collective_compute(
                "AllReduce",
                ALU.add,
                replica_groups=[list(range(8))],
                ins=[ar_in_d[:]],
                outs=[ar_out_d[:]],
            )
            gstats_t = bigpool.tile([R, 2], F32)
            nc.sync.dma_start(gstats_t[:], ar_out_d[:])

            # BN coefficients
            mu_t = bigpool.tile([R, 1], F32)
            nc.vector.tensor_scalar(mu_t[:], gstats_t[:, 0:1], 1.0 / N_TOT, None, ALU.mult)
            ex2_t = bigpool.tile([R, 1], F32)
            nc.vector.tensor_scalar(ex2_t[:], gstats_t[:, 1:2], 1.0 / N_TOT, None, ALU.mult)
            nvar_t = bigpool.tile([R, 1], F32)
            nc.vector.scalar_tensor_tensor(
                nvar_t[:], mu_t[:], mu_t[:], ex2_t[:], ALU.mult, ALU.subtract
            )  # mu^2 - E[y^2] = -var
            vpe_t = bigpool.tile([R, 1], F32)
            nc.vector.tensor_scalar(
                vpe_t[:], nvar_t[:], -1.0, 1e-5, ALU.mult, ALU.add
            )  # var + eps
            sd_t = bigpool.tile([R, 1], F32)
            nc.scalar.sqrt(sd_t[:], vpe_t[:])
            inv_t = bigpool.tile([R, 1], F32)
            nc.vector.reciprocal(inv_t[:], sd_t[:])
            gp_t = bigpool.tile([R, 1], F32)
            nc.vector.tensor_tensor(gp_t[:], gamma_t[:], inv_t[:], ALU.mult)
            mg_t = bigpool.tile([R, 1], F32)
            nc.vector.tensor_tensor(mg_t[:], mu_t[:], gp_t[:], ALU.mult)
            bp_t = bigpool.tile([R, 1], F32)
            nc.vector.tensor_tensor(bp_t[:], beta_t[:], mg_t[:], ALU.subtract)
            yn_t = bigpool.tile([R, 2 * S], F32)
            nc.vector.tensor_scalar(
                yn_t[:], y_t[:], gp_t[:], bp_t[:], ALU.mult, ALU.add
            )
            yr_t = bigpool.tile([R, 2 * S], F32R)
            nc.scalar.activation(yr_t[:], yn_t[:], ACTF.Relu)

            # sh: [H, S] then per-head broadcast
            psh = pspool.tile([H, S], F32, tag="ps")
            nc.tensor.matmul(psh[:], convhT_t[:], yr_t[:, :S], start=True, stop=True)
            nc.scalar.activation(sh_t[:], psh[:], ACTF.Sigmoid)
            # sw transposed: per s-tile [128, H], then *8
            for st in range(KT):
                psw = pspool.tile([128, H], F32, tag="ps")
                nc.tensor.matmul(
                    psw[:],
                    yr_t[:, S + st * 128 : S + (st + 1) * 128],
                    convwT_t[:],
                    start=True,
                    stop=True,
                )
                sg = wk2pool.tile([128, H], F32, tag="swsig")
                nc.scalar.activation(sg[:], psw[:], ACTF.Sigmoid)
                nc.vector.tensor_scalar(
                    sw8_cols[:, st, :], sg[:], 8.0, None, ALU.mult
                )

            # =========== PHASE C: bias + softmax + context ===========
            for h in range(H):
                jh, p0 = divmod(h * D, 128)
                sh_stage = wk2pool.tile([1, S], F16, tag="shstage")
                nc.gpsimd.dma_start(sh_stage[:], sh_t[h : h + 1, :])
                shb = pspool.tile([128, S], F32, tag="ps")
                nc.tensor.matmul(shb[:], ones_row16[:], sh_stage[:], start=True, stop=True)
                pctx = pspool.tile([65, S], F32, tag="ps")
                for kt in range(KT):
                    s16 = s16_t[:, h * KT + kt, :]
                    tmp = wk3pool.tile([128, S], F16, tag="ctmp")
                    nc.vector.scalar_tensor_tensor(
                        tmp[:], s16, 8.0, shb[:], ALU.is_ge, ALU.mult
                    )
                    pre = wk3pool.tile([128, S], F16, tag="cpre")
                    nc.vector.scalar_tensor_tensor(
                        pre[:],
                        tmp[:],
                        sw8_cols[:, kt, h : h + 1],
                        s16,
                        ALU.mult,
                        ALU.add,
                    )
                    e_t = epool.tile([128, S], BF16, tag="ebuf")
                    nc.scalar.activation(
                        e_t[:],
                        pre[:],
                        ACTF.Exp,
                        bias=maskT_t[:, kt : kt + 1],
                        scale=0.125,
                    )
                    nc.tensor.matmul(
                        pctx[:],
                        v_t[:, kt, h * 65 : (h + 1) * 65],
                        e_t[:],
                        start=(kt == 0),
                        stop=(kt == KT - 1),
                    )
                r_row = wk2pool.tile([1, S], F32R, tag="rrow")
                with nc.allow_low_precision(reason="softmax denom reciprocal at fp32r"):
                    nc.vector.reciprocal(r_row[:], pctx[64:65, :])
                r_b = pspool.tile([D, S], F32, tag="ps")
                nc.tensor.matmul(r_b[:], ones_row_r[:], r_row[:], start=True, stop=True)
                r_sb = wk2pool.tile([D, S], F32, tag="rsb")
                nc.scalar.copy(r_sb[:], r_b[:])
                outp = wk2pool.tile([D, S], F32, tag="outp")
                nc.vector.tensor_tensor(outp[:], pctx[0:D, :], r_sb[:], ALU.mult)
                nc.sync.dma_start(outT_d[h * D : (h + 1) * D, :], outp[:])

    nc.compile()
    return nc


def _prep_inputs(
    hidden_states,
    attention_mask,
    Wq,
    bq,
    Wk,
    bk,
    Wv,
    bv,
    conv1_w,
    bn_gamma,
    bn_beta,
    convh_w,
    convw_w,
):
    f32 = np.float32

    def pad_w(W, b):
        Wp = np.zeros((NI * 128, DM), f32)
        Wp[:DM] = _round_fp32r(np.asarray(W, f32))
        Wp[DM] = _round_fp32r(np.asarray(b, f32))
        return Wp

    Wqp = pad_w(Wq, bq)
    Wkp = pad_w(Wk, bk)
    Wvp = pad_w(Wv, bv)
    wq_p = np.ascontiguousarray(
        Wqp.reshape(NI, 128, NT, 128).transpose(2, 0, 1, 3)
    )
    wk_p = np.ascontiguousarray(
        Wkp.reshape(NI, 128, NT, 128).transpose(2, 0, 1, 3)
    )
    wv_p = np.ascontiguousarray(Wvp.reshape(NI, 128, DM))
    conv1T = np.ascontiguousarray(_round_fp32r(np.asarray(conv1_w, f32).T / S))
    convhT = np.ascontiguousarray(_round_fp32r(np.asarray(convh_w, f32).T))
    convwT = np.ascontiguousarray(_round_fp32r(np.asarray(convw_w, f32).T))
    gamma = np.asarray(bn_gamma, f32).reshape(R, 1)
    beta = np.asarray(bn_beta, f32).reshape(R, 1)

    hs = np.asarray(hidden_states, f32)
    am = np.asarray(attention_mask, f32)
    in_maps = []
    for b in range(B):
        hsT = np.zeros((NI * 128, S), f32)
        hsT[:DM] = _round_fp32r(hs[b].T)
        hsT[DM] = 1.0
        maskT = np.ascontiguousarray(am[b, 0, 0].reshape(KT, 128).T)
        in_maps.append(
            dict(
                hsT=hsT,
                wq=wq_p,
                wk=wk_p,
                wv=wv_p,
                maskT=maskT,
                conv1T=conv1T,
                convhT=convhT,
                convwT=convwT,
                gamma=gamma,
                beta=beta,
            )
        )
    return in_maps


def _run(inputs, trace=False, trace_kwargs=None):
    if "nc" not in _CACHE:
        _CACHE["nc"] = _build()
    nc = _CACHE["nc"]
    in_maps = _prep_inputs(**inputs)
    res = run_bass_kernel_spmd(
        nc, in_maps, list(range(8)), trace=trace, **(trace_kwargs or {})
    )
    out = np.stack([np.ascontiguousarray(r["outT"].T) for r in res.results])
    return out, res


def kernel(**inputs):
    out, _ = _run(inputs, trace=False)
    return out
